# revision 28
# baseline (speedup 1.0000x reference)
"""Trainium2 Bass kernel for LongRangeTCN (4-layer dilated causal conv + BN + LIF + residual).

Sharding: data-parallel over batch B=32 -> 4 per core across 8 NeuronCores.
Per core layout (SBUF, fp32):
  X   [128, 4, 4112]  residual/input; cols [0,16) zero pad (conv halo), col 16+t = x_t
  XH  [128, 4, 4160]  scan input/trajectory; cols [0,64) zero (warmup), col 64+t holds
                      xh_t = 0.5*BN(conv(x))_t, overwritten in-place by A_t during the scan
  WT/WL [128, 4, 3, 128] folded conv weights split as fp32r-exact hi + remainder
  BIAS [128, 4]        folded BN bias (per-channel) * 0.5

Conv: fp32r matmuls (4x PE throughput vs fp32; fp32r rounds inputs to 11
mantissa bits, so W is split host-side into Wh + Wl and both accumulated,
leaving only activation rounding). PE is pre-warmed with dummy matmuls so the
first conv doesn't run at the cold p-state. 6 matmuls/chunk accumulate in
PSUM; ACT evacuates adding the BN bias.

LIF scan: v' = (A<1)*A with A = 0.5*v + xh_t, as 128 parallel chunks of 32
steps per batch with a 16-step warmup (conv fp32r noise ~1e-4 dominates the
0.5^16 chunk-carry error). Chunks are split into two interleaved groups so
consecutive DVE ops are independent: the engine's ~95ns write-to-read drain
(which hardware requires a semaphore wait for) lands while the other group's
op executes. Spikes s=(A>=1) + residual are fused full-width ops.
"""

import numpy as np

TAU, VTH, EPS, K = 2.0, 1.0, 1e-5, 3
DILATIONS = (1, 2, 4, 8)
B, C, T = 32, 128, 4096
NCORES = 8
BL = B // NCORES          # 4 batches per core
H = 12                    # scan warmup steps
LSC = 32                  # scan chunk length
NC2 = T // LSC            # 128 chunks per batch
NG = NC2 // 2             # chunks per interleave group
PAD0 = 64                 # zero-pad columns at the head of each batch row in XH
OFF = PAD0 - H            # 48: step j of chunk c touches col c*LSC + OFF + j
PADX = 16                 # conv left halo (max (K-1)*d = 16)
SX = PADX + T             # 4112
SXH = PAD0 + T            # 4160 = 130*32
NS = 2                    # batch streams per core (pipeline conv under scan)
BS = BL // NS             # 2 batches per stream
TH = T // 2               # head/tail DMA split

_cache = {}


def _build():
    import concourse.bass as bass
    import concourse.bacc as bacc
    import concourse.tile as tile
    import concourse.mybir as mybir

    dt = mybir.dt.float32
    f32r = mybir.dt.float32r
    Alu = mybir.AluOpType
    Act = mybir.ActivationFunctionType

    nc = bacc.Bacc("TRN2", target_bir_lowering=False, debug=False)
    x_d = nc.dram_tensor("x", [BL, C, T], dt, kind="ExternalInput")
    wt_d = nc.dram_tensor("wt", [C, 4, K, C], dt, kind="ExternalInput")
    wl_d = nc.dram_tensor("wl", [C, 4, K, C], dt, kind="ExternalInput")
    b_d = nc.dram_tensor("bias", [C, 4], dt, kind="ExternalInput")
    o_d = nc.dram_tensor("out", [BL, C, T], dt, kind="ExternalOutput")

    # Scan state lives outside tile pools; the only cross-engine edge into it
    # (ACT evict -> DVE scan) gets one manual semaphore per stream/layer.
    XH = nc.alloc_sbuf_tensor("XHraw", [C, BL, SXH], dt).ap()
    V = [nc.alloc_sbuf_tensor(f"Vraw{s}", [C, BS, NC2], dt).ap() for s in range(NS)]
    SCR = [nc.alloc_sbuf_tensor(f"SCRraw{s}", [C, BS, NC2], dt).ap() for s in range(NS)]
    WU = nc.alloc_sbuf_tensor("WUraw", [C, 640], dt).ap()  # PE warmup garbage
    ev_sem = nc.alloc_semaphore("evict_done")

    with tile.TileContext(nc) as tc:
        with (
            tc.tile_pool(name="big", bufs=1) as big,
            tc.tile_pool(name="small", bufs=1) as small,
            tc.tile_pool(name="psum", bufs=4, space="PSUM") as pp,
        ):
            X = big.tile([C, BL, SX], dt, tag="X")
            WT = small.tile([C, 4, K, C], dt, tag="WT")
            WL = small.tile([C, 4, K, C], dt, name="WL", tag="WL")
            BIAS = small.tile([C, 4], dt, tag="BIAS")

            # p-state warmup: keep PE continuously busy through the input DMA
            # window so real convs start at full clock.
            wups = pp.tile([C, 512], dt, tag="wup")
            for _ in range(20):
                nc.tensor.matmul(wups[:], WU[:, 0:128].bitcast(f32r),
                                 WU[:, 128:640].bitcast(f32r), start=True, stop=True)

            # layer-0 weights first so only they gate the first conv
            nc.sync.dma_start(WT[:, 0].bitcast(f32r), wt_d[:, 0].bitcast(f32r))
            nc.sync.dma_start(WL[:, 0].bitcast(f32r), wl_d[:, 0].bitcast(f32r))
            nc.sync.dma_start(BIAS[:], b_d[:])
            nc.vector.memset(X[:, :, 0:PADX], 0.0)
            nc.vector.memset(XH[:, :, 0:PAD0], 0.0)
            for s in range(NS):
                nc.vector.memset(V[s], 0.0)
            TQ = T // 4
            for b in range(BL):
                for hh in range(4):
                    nc.sync.dma_start(
                        X[:, b, PADX + hh * TQ : PADX + (hh + 1) * TQ].bitcast(f32r),
                        x_d[b][:, hh * TQ : (hh + 1) * TQ].bitcast(f32r))
                if b == 1:
                    # remaining layers' weights after stream 0's activations
                    nc.sync.dma_start(WT[:, 1:4].bitcast(f32r), wt_d[:, 1:4].bitcast(f32r))
                    nc.sync.dma_start(WL[:, 1:4].bitcast(f32r), wl_d[:, 1:4].bitcast(f32r))

            XH4 = XH.rearrange("p a (c l) -> p a c l", l=LSC)
            ev_cnt = [0]

            def conv(li, d, s):
                for b in range(s * BS, (s + 1) * BS):
                    for t0 in range(0, T, 512):
                        ps = pp.tile([C, 512], dt, tag="ps")
                        for k in range(K):
                            sh = (K - 1 - k) * d
                            rhs = X[:, b, PADX + t0 - sh : PADX + t0 - sh + 512].bitcast(f32r)
                            nc.tensor.matmul(ps[:], WT[:, li, k, :].bitcast(f32r), rhs,
                                             start=(k == 0), stop=False)
                            nc.tensor.matmul(ps[:], WL[:, li, k, :].bitcast(f32r), rhs,
                                             start=False, stop=(k == K - 1))
                        nc.scalar.activation(
                            XH[:, b, PAD0 + t0 : PAD0 + t0 + 512], ps[:],
                            Act.Identity, bias=BIAS[:, li : li + 1], scale=1.0,
                        )
                nc.scalar.sem_inc(ev_sem)
                ev_cnt[0] += 1

            def scan(s):
                b0 = s * BS
                last = H + LSC - 1
                for j in range(H + LSC):
                    # absolute col of step j in chunk c is c*LSC + OFF + j;
                    # OFF+j spans [48, 96) so the chunk-view offset is 1 or 2
                    qo, r = divmod(OFF + j, LSC)
                    cols = [XH4[:, b0 : b0 + BS, g * NG + qo : (g + 1) * NG + qo, r]
                            for g in range(2)]
                    ads = [SCR[s][:, :, g * NG : (g + 1) * NG] if j < H else cols[g]
                           for g in range(2)]
                    vs = [V[s][:, :, g * NG : (g + 1) * NG] for g in range(2)]
                    for g in range(2):
                        # A = 0.5*v + xh_t (overwrites xh col in place when j>=H)
                        op = nc.vector.scalar_tensor_tensor(
                            ads[g], vs[g], 0.5, cols[g], op0=Alu.mult, op1=Alu.add)
                        if j == 0 and g == 0:
                            op.wait_op(ev_sem, ev_cnt[0], "sem-ge")
                    # v' = (A < 1) * A; the final step's state is never used
                    if j != last:
                        for g in range(2):
                            nc.vector.scalar_tensor_tensor(
                                vs[g], ads[g], float(VTH), ads[g],
                                op0=Alu.is_lt, op1=Alu.mult)

            def spike_res(s, t0, t1):
                b0 = s * BS
                nc.vector.scalar_tensor_tensor(
                    X[:, b0 : b0 + BS, PADX + t0 : PADX + t1].bitcast(f32r),
                    XH[:, b0 : b0 + BS, PAD0 + t0 : PAD0 + t1], float(VTH),
                    X[:, b0 : b0 + BS, PADX + t0 : PADX + t1],
                    op0=Alu.is_ge, op1=Alu.add)

            # software pipeline: stream s+1's conv (PE) overlaps stream s's
            # scan (DVE); across layers likewise — Tile schedules by deps.
            for li, d in enumerate(DILATIONS):
                for s in range(NS):
                    conv(li, d, s)
                    scan(s)
                    if li < len(DILATIONS) - 1:
                        spike_res(s, 0, T)
                    else:
                        # quarter the final spike+store so output DMA overlaps
                        for hh in range(4):
                            q0, q1 = hh * (T // 4), (hh + 1) * (T // 4)
                            spike_res(s, q0, q1)
                            for b in range(s * BS, (s + 1) * BS):
                                nc.sync.dma_start(
                                    o_d[b][:, q0:q1], X[:, b, PADX + q0 : PADX + q1])

    nc.clear_and_free_semaphores([ev_sem])
    nc.all_engine_barrier()
    nc.compile()
    return nc


def _round11(a):
    """Round fp32 to 11 explicit mantissa bits (fp32r's internal rounding)."""
    u = np.asarray(a, np.float32).view(np.uint32)
    return ((u + np.uint32(1 << 11)) & np.uint32(0xFFFFF000)).view(np.float32)


def kernel(x, w, gamma, beta, mean, var, **_):
    from concourse.bass_utils import run_bass_kernel_spmd

    x = np.ascontiguousarray(x, np.float32)
    inv = (gamma / np.sqrt(var + EPS)).astype(np.float32)          # [4, C]
    # wt[ci, l, k, co] = 0.5 * w[l, co, ci, k] * inv[l, co]
    wt = (0.5 * w * inv[:, :, None, None]).astype(np.float32)      # [4, Co, Ci, K]
    wt = np.ascontiguousarray(wt.transpose(2, 0, 3, 1))            # [Ci, 4, K, Co]
    wh = _round11(wt)                                              # exact under fp32r
    wl = np.ascontiguousarray(wt - wh)                             # remainder term
    bias = (0.5 * (beta - mean * inv)).astype(np.float32).T        # [C, 4]
    bias = np.ascontiguousarray(bias)

    if "nc" not in _cache:
        _cache["nc"] = _build()
    nc = _cache["nc"]

    in_maps = [
        {"x": np.ascontiguousarray(x[i * BL : (i + 1) * BL]), "wt": wh, "wl": wl,
         "bias": bias}
        for i in range(NCORES)
    ]
    res = run_bass_kernel_spmd(nc, in_maps, list(range(NCORES)))
    return np.concatenate([res.results[i]["out"] for i in range(NCORES)], axis=0)


# revision 29
# speedup vs baseline: 1.0022x; 1.0022x over previous
"""Trainium2 Bass kernel for LongRangeTCN (4-layer dilated causal conv + BN + LIF + residual).

Sharding: data-parallel over batch B=32 -> 4 per core across 8 NeuronCores.
Per core layout (SBUF, fp32):
  X   [128, 4, 4112]  residual/input; cols [0,16) zero pad (conv halo), col 16+t = x_t
  XH  [128, 4, 4160]  scan input/trajectory; cols [0,64) zero (warmup), col 64+t holds
                      xh_t = 0.5*BN(conv(x))_t, overwritten in-place by A_t during the scan
  WT/WL [128, 4, 3, 128] folded conv weights split as fp32r-exact hi + remainder
  BIAS [128, 4]        folded BN bias (per-channel) * 0.5

Conv: fp32r matmuls (4x PE throughput vs fp32; fp32r rounds inputs to 11
mantissa bits, so W is split host-side into Wh + Wl and both accumulated,
leaving only activation rounding). PE is pre-warmed with dummy matmuls so the
first conv doesn't run at the cold p-state. 6 matmuls/chunk accumulate in
PSUM; ACT evacuates adding the BN bias.

LIF scan: v' = (A<1)*A with A = 0.5*v + xh_t, as 128 parallel chunks of 32
steps per batch with a 16-step warmup (conv fp32r noise ~1e-4 dominates the
0.5^16 chunk-carry error). Chunks are split into two interleaved groups so
consecutive DVE ops are independent: the engine's ~95ns write-to-read drain
(which hardware requires a semaphore wait for) lands while the other group's
op executes. Spikes s=(A>=1) + residual are fused full-width ops.
"""

import numpy as np

TAU, VTH, EPS, K = 2.0, 1.0, 1e-5, 3
DILATIONS = (1, 2, 4, 8)
B, C, T = 32, 128, 4096
NCORES = 8
BL = B // NCORES          # 4 batches per core
H = 12                    # scan warmup steps
LSC = 32                  # scan chunk length
NC2 = T // LSC            # 128 chunks per batch
NG = NC2 // 2             # chunks per interleave group
PAD0 = 64                 # zero-pad columns at the head of each batch row in XH
OFF = PAD0 - H            # 48: step j of chunk c touches col c*LSC + OFF + j
PADX = 16                 # conv left halo (max (K-1)*d = 16)
SX = PADX + T             # 4112
SXH = PAD0 + T            # 4160 = 130*32
NS = 2                    # batch streams per core (pipeline conv under scan)
BS = BL // NS             # 2 batches per stream
TH = T // 2               # head/tail DMA split

_cache = {}


def _build():
    import concourse.bass as bass
    import concourse.bacc as bacc
    import concourse.tile as tile
    import concourse.mybir as mybir

    dt = mybir.dt.float32
    f32r = mybir.dt.float32r
    Alu = mybir.AluOpType
    Act = mybir.ActivationFunctionType

    nc = bacc.Bacc("TRN2", target_bir_lowering=False, debug=False)
    x_d = nc.dram_tensor("x", [BL, C, T], dt, kind="ExternalInput")
    wt_d = nc.dram_tensor("wt", [C, 4, K, C], dt, kind="ExternalInput")
    wl_d = nc.dram_tensor("wl", [C, 4, K, C], dt, kind="ExternalInput")
    b_d = nc.dram_tensor("bias", [C, 4], dt, kind="ExternalInput")
    o_d = nc.dram_tensor("out", [BL, C, T], dt, kind="ExternalOutput")

    # Scan state lives outside tile pools; the only cross-engine edge into it
    # (ACT evict -> DVE scan) gets one manual semaphore per stream/layer.
    XH = nc.alloc_sbuf_tensor("XHraw", [C, BL, SXH], dt).ap()
    V = [nc.alloc_sbuf_tensor(f"Vraw{s}", [C, BS, NC2], dt).ap() for s in range(NS)]
    SCR = [nc.alloc_sbuf_tensor(f"SCRraw{s}", [C, BS, NC2], dt).ap() for s in range(NS)]
    WU = nc.alloc_sbuf_tensor("WUraw", [C, 640], dt).ap()  # PE warmup garbage
    ev_sem = nc.alloc_semaphore("evict_done")

    with tile.TileContext(nc) as tc:
        with (
            tc.tile_pool(name="big", bufs=1) as big,
            tc.tile_pool(name="small", bufs=1) as small,
            tc.tile_pool(name="psum", bufs=4, space="PSUM") as pp,
        ):
            X = big.tile([C, BL, SX], dt, tag="X")
            WT = small.tile([C, 4, K, C], dt, tag="WT")
            WL = small.tile([C, 4, K, C], dt, name="WL", tag="WL")
            BIAS = small.tile([C, 4], dt, tag="BIAS")

            # p-state warmup: keep PE continuously busy through the input DMA
            # window so real convs start at full clock.
            wups = pp.tile([C, 512], dt, tag="wup")
            for _ in range(20):
                nc.tensor.matmul(wups[:], WU[:, 0:128].bitcast(f32r),
                                 WU[:, 128:640].bitcast(f32r), start=True, stop=True)

            # layer-0 weights first so only they gate the first conv
            nc.sync.dma_start(WT[:, 0].bitcast(f32r), wt_d[:, 0].bitcast(f32r))
            nc.sync.dma_start(WL[:, 0].bitcast(f32r), wl_d[:, 0].bitcast(f32r))
            nc.sync.dma_start(BIAS[:], b_d[:])
            nc.vector.memset(X[:, :, 0:PADX], 0.0)
            nc.vector.memset(XH[:, :, 0:PAD0], 0.0)
            for s in range(NS):
                nc.vector.memset(V[s], 0.0)
            TQ = T // 4
            for b in range(BL):
                for hh in range(4):
                    nc.sync.dma_start(
                        X[:, b, PADX + hh * TQ : PADX + (hh + 1) * TQ].bitcast(f32r),
                        x_d[b][:, hh * TQ : (hh + 1) * TQ].bitcast(f32r))
                if b == 1:
                    # remaining layers' weights after stream 0's activations
                    nc.sync.dma_start(WT[:, 1:4].bitcast(f32r), wt_d[:, 1:4].bitcast(f32r))
                    nc.sync.dma_start(WL[:, 1:4].bitcast(f32r), wl_d[:, 1:4].bitcast(f32r))

            XH4 = XH.rearrange("p a (c l) -> p a c l", l=LSC)
            ev_cnt = [0]

            def conv(li, d, s):
                for b in range(s * BS, (s + 1) * BS):
                    for t0 in range(0, T, 512):
                        ps = pp.tile([C, 512], dt, tag="ps")
                        for k in range(K):
                            sh = (K - 1 - k) * d
                            rhs = X[:, b, PADX + t0 - sh : PADX + t0 - sh + 512].bitcast(f32r)
                            nc.tensor.matmul(ps[:], WT[:, li, k, :].bitcast(f32r), rhs,
                                             start=(k == 0), stop=False)
                            nc.tensor.matmul(ps[:], WL[:, li, k, :].bitcast(f32r), rhs,
                                             start=False, stop=(k == K - 1))
                        nc.scalar.activation(
                            XH[:, b, PAD0 + t0 : PAD0 + t0 + 512], ps[:],
                            Act.Identity, bias=BIAS[:, li : li + 1], scale=1.0,
                        )
                nc.scalar.sem_inc(ev_sem)
                ev_cnt[0] += 1

            def scan(s):
                b0 = s * BS
                last = H + LSC - 1
                for j in range(H + LSC):
                    # absolute col of step j in chunk c is c*LSC + OFF + j;
                    # OFF+j spans [48, 96) so the chunk-view offset is 1 or 2
                    qo, r = divmod(OFF + j, LSC)
                    cols = [XH4[:, b0 : b0 + BS, g * NG + qo : (g + 1) * NG + qo, r]
                            for g in range(2)]
                    ads = [SCR[s][:, :, g * NG : (g + 1) * NG] if j < H else cols[g]
                           for g in range(2)]
                    vs = [V[s][:, :, g * NG : (g + 1) * NG] for g in range(2)]
                    for g in range(2):
                        # A = 0.5*v + xh_t (overwrites xh col in place when j>=H)
                        op = nc.vector.scalar_tensor_tensor(
                            ads[g], vs[g], 0.5, cols[g], op0=Alu.mult, op1=Alu.add)
                        if j == 0 and g == 0:
                            op.wait_op(ev_sem, ev_cnt[0], "sem-ge")
                    # v' = (A < 1) * A; the final step's state is never used
                    if j != last:
                        for g in range(2):
                            nc.vector.scalar_tensor_tensor(
                                vs[g], ads[g], float(VTH), ads[g],
                                op0=Alu.is_lt, op1=Alu.mult)

            def spike_res(s, t0, t1):
                b0 = s * BS
                nc.vector.scalar_tensor_tensor(
                    X[:, b0 : b0 + BS, PADX + t0 : PADX + t1].bitcast(f32r),
                    XH[:, b0 : b0 + BS, PAD0 + t0 : PAD0 + t1], float(VTH),
                    X[:, b0 : b0 + BS, PADX + t0 : PADX + t1],
                    op0=Alu.is_ge, op1=Alu.add)

            # software pipeline: stream s+1's conv (PE) overlaps stream s's
            # scan (DVE); across layers likewise — Tile schedules by deps.
            for li, d in enumerate(DILATIONS):
                for s in range(NS):
                    conv(li, d, s)
                    scan(s)
                    if li < len(DILATIONS) - 1:
                        spike_res(s, 0, T)
                    else:
                        # split the final spike+store so output DMA overlaps
                        for hh in range(8):
                            q0, q1 = hh * (T // 8), (hh + 1) * (T // 8)
                            spike_res(s, q0, q1)
                            for b in range(s * BS, (s + 1) * BS):
                                nc.sync.dma_start(
                                    o_d[b][:, q0:q1], X[:, b, PADX + q0 : PADX + q1])

    nc.clear_and_free_semaphores([ev_sem])
    nc.all_engine_barrier()
    nc.compile()
    return nc


def _round11(a):
    """Round fp32 to 11 explicit mantissa bits (fp32r's internal rounding)."""
    u = np.asarray(a, np.float32).view(np.uint32)
    return ((u + np.uint32(1 << 11)) & np.uint32(0xFFFFF000)).view(np.float32)


def kernel(x, w, gamma, beta, mean, var, **_):
    from concourse.bass_utils import run_bass_kernel_spmd

    x = np.ascontiguousarray(x, np.float32)
    inv = (gamma / np.sqrt(var + EPS)).astype(np.float32)          # [4, C]
    # wt[ci, l, k, co] = 0.5 * w[l, co, ci, k] * inv[l, co]
    wt = (0.5 * w * inv[:, :, None, None]).astype(np.float32)      # [4, Co, Ci, K]
    wt = np.ascontiguousarray(wt.transpose(2, 0, 3, 1))            # [Ci, 4, K, Co]
    wh = _round11(wt)                                              # exact under fp32r
    wl = np.ascontiguousarray(wt - wh)                             # remainder term
    bias = (0.5 * (beta - mean * inv)).astype(np.float32).T        # [C, 4]
    bias = np.ascontiguousarray(bias)

    if "nc" not in _cache:
        _cache["nc"] = _build()
    nc = _cache["nc"]

    in_maps = [
        {"x": np.ascontiguousarray(x[i * BL : (i + 1) * BL]), "wt": wh, "wl": wl,
         "bias": bias}
        for i in range(NCORES)
    ]
    res = run_bass_kernel_spmd(nc, in_maps, list(range(NCORES)))
    return np.concatenate([res.results[i]["out"] for i in range(NCORES)], axis=0)
